# revision 11
# baseline (speedup 1.0000x reference)
"""Class-aware regularization loss kernel for Trainium2 (8 NeuronCores).

Contract: kernel(features[8,512,128,128] f32, label[8,128,128] i32) -> [2] f32
  out[0] = intra-class squared-distance-to-center loss
  out[1] = inter-class center-to-center (c2c) loss

Sharding: data-parallel over batch B=8, one batch per core. Each core
computes sufficient statistics for its batch from feats [D=512, HW=16384]:
  class_sum[c, d] = sum_{n : lab[n]==c} feats[n, d]   (one-hot matmul on PE)
  sumsq           = sum_n ||feats[n]||^2              (ScalarE square+accum)
The host combines: counts via bincount, per-batch centers, the c2c loss on
batch-mean centers, and the algebraically-expanded intra loss
  intra*total = sumsq + sum_{b,c} ||S_bc||^2 * (n/(n+eps)^2 - 2/(n+eps))
which avoids any per-pixel center gather on device.

Device pipeline per core (memory-bound; ~94us HBM floor for 32 MiB fp32):
  HWDGE DMA streams feats fp32 into SBUF in 0.5 MiB chunks (deep
  double-buffering); ScalarE squares+accumulates the fp32 tiles for sumsq
  while GPSIMD casts them to bf16 in parallel; PE transposes 128x128 bf16
  blocks (d-major -> pixel-major) into PSUM (1 cyc/row in bf16); DVE copies
  grouped [128, 2048] PSUM tiles back to SBUF; PE accumulates
  one-hot.T @ featsT into a [21,512] PSUM bank over all 128 pixel-tiles.
  The one-hot is built on-chip from labels (cast -> PE transpose -> 21
  is_equal compares). Small chunks keep PE busy-gaps under the ~3.4us HAM
  re-throttle window.
"""

import os
import sys

for _p in ("/opt/trn_rl_repo",):
    if _p not in sys.path and os.path.isdir(_p):
        sys.path.insert(0, _p)

from contextlib import ExitStack

import numpy as np

import concourse.bacc as bacc
import concourse.bass as bass
import concourse.mybir as mybir
import concourse.tile as tile
from concourse.bass_utils import run_bass_kernel_spmd
from concourse.masks import make_identity

F32 = mybir.dt.float32
BF16 = mybir.dt.bfloat16
I32 = mybir.dt.int32

B = 8
D = 512  # feature channels
HW = 128 * 128  # pixels per batch
C = 21  # classes
IGNORE = 255
EPS = 1e-6
C2C_THR = 0.5

P = 128
NDB = D // P  # 4 d-blocks


def build_nc(hw: int = HW, mode: str = "poolcast", group: int = 4, chunk: int = 1024,
             fbufs: int = 16, reps: int = 1):
    """Build the single-core Bass program (run SPMD across 8 cores).

    mode "dmacast": Pool/SWDGE DMA converts fp32->bf16 during the load.
    mode "poolcast": HWDGE fp32 DMA + GPSIMD cast pass (fallback).
    reps > 1 repeats the whole body (for wall-clock timing of the kernel).
    """
    nt = hw // P
    nchunk = max(1, hw // chunk)
    chunk = hw // nchunk
    tpc = chunk // P
    assert tpc % group == 0

    nc = bacc.Bacc()
    feats_d = nc.dram_tensor("features", [D, hw], F32, kind="ExternalInput")
    label_d = nc.dram_tensor("label", [hw], I32, kind="ExternalInput")
    cs_d = nc.dram_tensor("class_sum", [C, D], F32, kind="ExternalOutput")
    ss_d = nc.dram_tensor("sumsq", [P, 1], F32, kind="ExternalOutput")

    feats_r = feats_d.rearrange("(db p) n -> db p n", p=P)
    label_r = label_d.rearrange("(j q) -> j q", q=P)  # [nt, 128]: tile j, pixel q

    with ExitStack() as ctx:
        tc = ctx.enter_context(tile.TileContext(nc))
        singles = ctx.enter_context(tc.tile_pool(name="singles", bufs=1))
        fpool = ctx.enter_context(tc.tile_pool(name="feats", bufs=fbufs))
        tpsum = ctx.enter_context(tc.tile_pool(name="tpsum", bufs=2, space="PSUM"))
        lpsum = ctx.enter_context(tc.tile_pool(name="lpsum", bufs=1, space="PSUM"))
        cpsum = ctx.enter_context(tc.tile_pool(name="cpsum", bufs=1, space="PSUM"))
        ftpool = ctx.enter_context(tc.tile_pool(name="featsT", bufs=3))

        ident_f = singles.tile([P, P], F32)
        make_identity(nc, ident_f[:])
        ident_b = singles.tile([P, P], BF16)
        nc.vector.tensor_copy(ident_b[:], ident_f[:])

        # ---- labels -> transposed fp32 -> one-hot (bf16) ----
        # lab_nat[j, q] = label[128j + q]. Transpose so partitions = pixel q.
        lab_i = singles.tile([nt, P], I32)
        nc.sync.dma_start(lab_i[:], label_r[:, :])
        lab_f = singles.tile([nt, P], F32)
        nc.vector.tensor_copy(lab_f[:], lab_i[:])
        lab_ps = lpsum.tile([P, nt], F32)
        nc.tensor.transpose(lab_ps[:], lab_f[:], ident_f[:nt, :nt])
        labT = singles.tile([P, nt], F32)
        nc.vector.tensor_copy(labT[:], lab_ps[:])
        # oh[q, j, c] = (labT[q, j] == c)
        oh = singles.tile([P, nt, C], BF16)
        for c in range(C):
            nc.vector.tensor_scalar(
                oh[:, :, c], labT[:], float(c), None, mybir.AluOpType.is_equal
            )

        stats = singles.tile([P, NDB * nchunk], F32)
        junk = singles.tile([P, chunk], BF16)
        psum_cs = cpsum.tile([C, D], F32)

        for _rep in range(reps):
            for ch in range(nchunk):
                btiles = []
                for db in range(NDB):
                    k = ch * NDB + db
                    src = feats_r[db, :, ch * chunk : (ch + 1) * chunk]
                    if mode == "dmacast":
                        ftb = fpool.tile([P, chunk], BF16, tag="ftb")
                        nc.gpsimd.dma_start(ftb[:], src)
                        sq_src = ftb
                    else:
                        ft = fpool.tile([P, chunk], F32, tag="ft")
                        nc.sync.dma_start(ft[:], src)
                        ftb = fpool.tile([P, chunk], BF16, tag="ftb")
                        nc.gpsimd.tensor_copy(ftb[:], ft[:])
                        sq_src = ft  # fp32: runs parallel to the cast, better precision
                    btiles.append(ftb)
                    # sumsq partial: accum_out = sum(square(sq_src))
                    nc.scalar.activation(
                        junk[:],
                        sq_src[:],
                        mybir.ActivationFunctionType.Square,
                        accum_out=stats[:, k : k + 1],
                    )
                for g in range(tpc // group):
                    pst = tpsum.tile([P, group * D], BF16, tag="pst")
                    for w in range(group):
                        jj = g * group + w
                        for db in range(NDB):
                            nc.tensor.transpose(
                                pst[:, w * D + db * P : w * D + (db + 1) * P],
                                btiles[db][:, jj * P : (jj + 1) * P],
                                ident_b[:],
                            )
                    ftT = ftpool.tile([P, group * D], BF16, tag="ftT")
                    nc.vector.tensor_copy(ftT[:], pst[:])
                    for w in range(group):
                        j = ch * tpc + g * group + w
                        nc.tensor.matmul(
                            psum_cs[:],
                            oh[:, j, :],
                            ftT[:, w * D : (w + 1) * D],
                            start=(j == 0),
                            stop=(j == nt - 1),
                        )

        # ---- outputs ----
        cs_sb = singles.tile([C, D], F32)
        nc.vector.tensor_copy(cs_sb[:], psum_cs[:])
        nc.sync.dma_start(cs_d[:, :], cs_sb[:])

        ss_sb = singles.tile([P, 1], F32)
        nc.vector.tensor_reduce(
            ss_sb[:], stats[:], axis=mybir.AxisListType.X, op=mybir.AluOpType.add
        )
        nc.sync.dma_start(ss_d[:, :], ss_sb[:])

    nc.compile()
    return nc


def _finalize(class_sums, sumsqs, labels):
    """Host combine: [B,21,512] f64 class sums, [B] f64 sumsq, [B,HW] labels."""
    lab = labels.reshape(B, -1)
    valid = lab != IGNORE
    counts = np.stack(
        [np.bincount(lab[b][valid[b]], minlength=C)[:C] for b in range(B)]
    ).astype(np.float64)  # [B, C]

    centers = class_sums / (counts[..., None] + EPS)  # [B, C, D]

    # inter-class c2c loss on batch-mean centers
    cm = centers.mean(axis=0)  # [C, D]
    cn = cm / np.maximum(np.linalg.norm(cm, axis=1, keepdims=True), 1e-12)
    sim = cn @ cn.T
    offdiag = 1.0 - np.eye(C)
    inter = (np.maximum(sim - C2C_THR, 0.0) * offdiag).sum() / (C + EPS)

    # intra loss via the ||f - center||^2 expansion
    s2 = (class_sums**2).sum(axis=2)  # [B, C] = ||S_bc||^2
    corr = s2 * (counts / (counts + EPS) ** 2 - 2.0 / (counts + EPS))
    total = valid.sum()
    intra = (sumsqs.sum() + corr.sum()) / (total + EPS)

    return np.array([intra, inter], dtype=np.float32)


def kernel(features, label, _nc=None, _raw=False):
    feats = np.ascontiguousarray(
        np.asarray(features, dtype=np.float32).reshape(B, D, HW)
    )
    labs = np.ascontiguousarray(np.asarray(label, dtype=np.int32).reshape(B, HW))

    nc = _nc if _nc is not None else build_nc()
    in_maps = [{"features": feats[b], "label": labs[b]} for b in range(B)]
    res = run_bass_kernel_spmd(nc, in_maps, list(range(B))).results

    class_sums = np.stack([res[b]["class_sum"] for b in range(B)]).astype(np.float64)
    sumsqs = np.array([res[b]["sumsq"].astype(np.float64).sum() for b in range(B)])

    return _finalize(class_sums, sumsqs, labs)


# revision 17
# speedup vs baseline: 1.0448x; 1.0448x over previous
"""Class-aware regularization loss kernel for Trainium2 (8 NeuronCores).

Contract: kernel(features[8,512,128,128] f32, label[8,128,128] i32) -> [2] f32
  out[0] = intra-class squared-distance-to-center loss
  out[1] = inter-class center-to-center (c2c) loss

Sharding: data-parallel over batch B=8, one batch per core. Each core
computes sufficient statistics for its batch from feats [D=512, HW=16384]:
  class_sum[c, d] = sum_{n : lab[n]==c} feats[n, d]   (one-hot matmul on PE)
  sumsq           = sum_n ||feats[n]||^2              (ScalarE square+accum)
The host combines: counts via bincount, per-batch centers, the c2c loss on
batch-mean centers, and the algebraically-expanded intra loss
  intra*total = sumsq + sum_{b,c} ||S_bc||^2 * (n/(n+eps)^2 - 2/(n+eps))
which avoids any per-pixel center gather on device.

Device pipeline per core (memory-bound; ~94us HBM floor for 32 MiB fp32):
  HWDGE DMA streams feats fp32 into SBUF in 0.5 MiB chunks (deep
  double-buffering); ScalarE squares+accumulates the fp32 tiles for sumsq
  while GPSIMD casts them to bf16 in parallel; PE transposes 128x128 bf16
  blocks (d-major -> pixel-major) into PSUM (1 cyc/row in bf16); DVE copies
  grouped [128, 2048] PSUM tiles back to SBUF; PE accumulates
  one-hot.T @ featsT into a [21,512] PSUM bank over all 128 pixel-tiles.
  The one-hot is built on-chip from labels (cast -> PE transpose -> 21
  is_equal compares). Small chunks keep PE busy-gaps under the ~3.4us HAM
  re-throttle window.
"""

import os
import sys

for _p in ("/opt/trn_rl_repo",):
    if _p not in sys.path and os.path.isdir(_p):
        sys.path.insert(0, _p)

from contextlib import ExitStack

import numpy as np

import concourse.bacc as bacc
import concourse.bass as bass
import concourse.mybir as mybir
import concourse.tile as tile
from concourse.bass_utils import run_bass_kernel_spmd
from concourse.masks import make_identity

F32 = mybir.dt.float32
BF16 = mybir.dt.bfloat16
I32 = mybir.dt.int32

B = 8
D = 512  # feature channels
HW = 128 * 128  # pixels per batch
C = 21  # classes
IGNORE = 255
EPS = 1e-6
C2C_THR = 0.5

P = 128
NDB = D // P  # 4 d-blocks


def build_nc(hw: int = HW, mode: str = "poolcast", group: int = 4, chunk: int = 1024,
             fbufs: int = 16, reps: int = 1):
    """Build the single-core Bass program (run SPMD across 8 cores).

    mode "dmacast": Pool/SWDGE DMA converts fp32->bf16 during the load.
    mode "poolcast": HWDGE fp32 DMA + GPSIMD cast pass to bf16.
    mode "fp32tp":  HWDGE fp32 DMA, fp32 PE transposes (2 cyc/row), DVE
        PSUM->SBUF copy does the fp32->bf16 cast; no separate cast pass.
    reps > 1 repeats the whole body (for wall-clock timing of the kernel).
    """
    nt = hw // P
    nchunk = max(1, hw // chunk)
    chunk = hw // nchunk
    tpc = chunk // P
    assert tpc % group == 0

    nc = bacc.Bacc()
    feats_d = nc.dram_tensor("features", [D, hw], F32, kind="ExternalInput")
    label_d = nc.dram_tensor("label", [hw], I32, kind="ExternalInput")
    cs_d = nc.dram_tensor("class_sum", [C, D], F32, kind="ExternalOutput")
    ss_d = nc.dram_tensor("sumsq", [P, 1], F32, kind="ExternalOutput")

    feats_r = feats_d.rearrange("(db p) n -> db p n", p=P)
    label_r = label_d.rearrange("(j q) -> j q", q=P)  # [nt, 128]: tile j, pixel q

    with ExitStack() as ctx:
        tc = ctx.enter_context(tile.TileContext(nc))
        singles = ctx.enter_context(tc.tile_pool(name="singles", bufs=1))
        fpool = ctx.enter_context(tc.tile_pool(name="feats", bufs=fbufs))
        tpsum = ctx.enter_context(tc.tile_pool(name="tpsum", bufs=2, space="PSUM"))
        lpsum = ctx.enter_context(tc.tile_pool(name="lpsum", bufs=1, space="PSUM"))
        cpsum = ctx.enter_context(tc.tile_pool(name="cpsum", bufs=1, space="PSUM"))
        ftpool = ctx.enter_context(tc.tile_pool(name="featsT", bufs=3))

        TPD = F32 if mode == "fp32tp" else BF16  # transpose datapath dtype

        ident_f = singles.tile([P, P], F32)
        make_identity(nc, ident_f[:])
        if mode == "fp32tp":
            ident_b = ident_f
        else:
            ident_b = singles.tile([P, P], BF16)
            nc.vector.tensor_copy(ident_b[:], ident_f[:])

        # ---- labels -> transposed fp32 -> one-hot (bf16) ----
        # lab_nat[j, q] = label[128j + q]. Transpose so partitions = pixel q.
        lab_i = singles.tile([nt, P], I32)
        nc.sync.dma_start(lab_i[:], label_r[:, :])
        lab_f = singles.tile([nt, P], F32)
        nc.vector.tensor_copy(lab_f[:], lab_i[:])
        lab_ps = lpsum.tile([P, nt], F32)
        nc.tensor.transpose(lab_ps[:], lab_f[:], ident_f[:nt, :nt])
        labT = singles.tile([P, nt], F32)
        nc.vector.tensor_copy(labT[:], lab_ps[:])
        # oh[q, j, c] = (labT[q, j] == c)
        oh = singles.tile([P, nt, C], BF16)
        for c in range(C):
            nc.vector.tensor_scalar(
                oh[:, :, c], labT[:], float(c), None, mybir.AluOpType.is_equal
            )

        stats = singles.tile([P, NDB * nchunk], F32)
        junk = singles.tile([P, chunk], BF16)
        psum_cs = cpsum.tile([C, D], F32)

        for _rep in range(reps):
            for ch in range(nchunk):
                btiles = []
                for db in range(NDB):
                    k = ch * NDB + db
                    src = feats_r[db, :, ch * chunk : (ch + 1) * chunk]
                    if mode == "dmacast":
                        ftb = fpool.tile([P, chunk], BF16, tag="ftb")
                        nc.gpsimd.dma_start(ftb[:], src)
                        sq_src = ftb
                    elif mode == "fp32tp":
                        ftb = fpool.tile([P, chunk], F32, tag="ft")
                        nc.sync.dma_start(ftb[:], src)
                        sq_src = ftb
                    else:
                        ft = fpool.tile([P, chunk], F32, tag="ft")
                        nc.sync.dma_start(ft[:], src)
                        ftb = fpool.tile([P, chunk], BF16, tag="ftb")
                        # split the bf16 cast between Pool and DVE so neither
                        # becomes a co-bottleneck with the DMA stream
                        cast_eng = nc.gpsimd if db % 2 == 0 else nc.vector
                        cast_eng.tensor_copy(ftb[:], ft[:])
                        sq_src = ft  # fp32: runs parallel to the cast, better precision
                    btiles.append(ftb)
                    # sumsq partial: accum_out = sum(square(sq_src))
                    nc.scalar.activation(
                        junk[:],
                        sq_src[:],
                        mybir.ActivationFunctionType.Square,
                        accum_out=stats[:, k : k + 1],
                    )
                for g in range(tpc // group):
                    pst = tpsum.tile([P, group * D], TPD, tag="pst")
                    for w in range(group):
                        jj = g * group + w
                        for db in range(NDB):
                            nc.tensor.transpose(
                                pst[:, w * D + db * P : w * D + (db + 1) * P],
                                btiles[db][:, jj * P : (jj + 1) * P],
                                ident_b[:],
                            )
                    ftT = ftpool.tile([P, group * D], BF16, tag="ftT")
                    nc.vector.tensor_copy(ftT[:], pst[:])
                    for w in range(group):
                        j = ch * tpc + g * group + w
                        nc.tensor.matmul(
                            psum_cs[:],
                            oh[:, j, :],
                            ftT[:, w * D : (w + 1) * D],
                            start=(j == 0),
                            stop=(j == nt - 1),
                        )

        # ---- outputs ----
        cs_sb = singles.tile([C, D], F32)
        nc.vector.tensor_copy(cs_sb[:], psum_cs[:])
        nc.sync.dma_start(cs_d[:, :], cs_sb[:])

        ss_sb = singles.tile([P, 1], F32)
        nc.vector.tensor_reduce(
            ss_sb[:], stats[:], axis=mybir.AxisListType.X, op=mybir.AluOpType.add
        )
        nc.sync.dma_start(ss_d[:, :], ss_sb[:])

    nc.compile()
    return nc


def _finalize(class_sums, sumsqs, labels):
    """Host combine: [B,21,512] f64 class sums, [B] f64 sumsq, [B,HW] labels."""
    lab = labels.reshape(B, -1)
    valid = lab != IGNORE
    counts = np.stack(
        [np.bincount(lab[b][valid[b]], minlength=C)[:C] for b in range(B)]
    ).astype(np.float64)  # [B, C]

    centers = class_sums / (counts[..., None] + EPS)  # [B, C, D]

    # inter-class c2c loss on batch-mean centers
    cm = centers.mean(axis=0)  # [C, D]
    cn = cm / np.maximum(np.linalg.norm(cm, axis=1, keepdims=True), 1e-12)
    sim = cn @ cn.T
    offdiag = 1.0 - np.eye(C)
    inter = (np.maximum(sim - C2C_THR, 0.0) * offdiag).sum() / (C + EPS)

    # intra loss via the ||f - center||^2 expansion
    s2 = (class_sums**2).sum(axis=2)  # [B, C] = ||S_bc||^2
    corr = s2 * (counts / (counts + EPS) ** 2 - 2.0 / (counts + EPS))
    total = valid.sum()
    intra = (sumsqs.sum() + corr.sum()) / (total + EPS)

    return np.array([intra, inter], dtype=np.float32)


_NC_CACHE = {}


def kernel(features, label, _nc=None, _raw=False):
    feats = np.ascontiguousarray(
        np.asarray(features, dtype=np.float32).reshape(B, D, HW)
    )
    labs = np.ascontiguousarray(np.asarray(label, dtype=np.int32).reshape(B, HW))

    nc = _nc
    if nc is None:
        if "default" not in _NC_CACHE:
            _NC_CACHE["default"] = build_nc()
        nc = _NC_CACHE["default"]
    in_maps = [{"features": feats[b], "label": labs[b]} for b in range(B)]
    res = run_bass_kernel_spmd(nc, in_maps, list(range(B))).results

    class_sums = np.stack([res[b]["class_sum"] for b in range(B)]).astype(np.float64)
    sumsqs = np.array([res[b]["sumsq"].astype(np.float64).sum() for b in range(B)])

    return _finalize(class_sums, sumsqs, labs)
